# revision 17
# baseline (speedup 1.0000x reference)
"""Single-head causal attention on 8 Trainium2 NeuronCores (Bass/Tile).

Problem: x [4, 4096, 1024] f32, Wq/Wk/Wv [64, 1024] f32 ->
         softmax(causal(q k^T * H^-0.5)) v   -> [4, 4096, 64] f32

Sharding: core = (batch b, parity p), b = core//2, p = core%2. Each core owns
the global 128-wide query tiles g = 2j+p (j=0..15) of its batch -- the parity
interleave balances causal work AND keeps the compiled graph identical across
all 8 cores (SPMD: one NEFF). All parity differences live in host-prepared
data (p=0 gets a zero-padded shifted x; a dbias input removes the pad's
exp(0)*128 contribution to every softmax denominator).

v2 design (from baseline trace analysis: 98.8us, PE 71% busy incl 19us
HAM-throttled, ScalarE 52us of ACTIVATE over 81 calls, 25us before the
first score exp, 12us serial epilogue tail):

  * Host pre-swizzles x^T per core into 5 wave-major DRAM tensors
    ([128, CT*wlen], 8KB contiguous lines) -> ONE DMA issue per wave
    (~0.7us) instead of 32 serialized 607ns issues; consumption-ordered.
  * ~40 warmup matmuls on a zero tile keep the PE HAM activity monitor
    at full clock (2.4GHz) across the initial DMA wait.
  * S^T dual-issue: the score matmul contracts over only H=64 features,
    so two k-tiles run CONCURRENTLY in the two 64-row halves of the PE
    array (tile_position row groups 0/64). K^T is duplicated into both
    partition halves (direct DVE evac to rows 64:128 + SBUF->SBUF DMA to
    rows 0:64); Q^T was already duplicated. 2x S^T throughput, exact.
  * Paired exp: one ACTIVATE per pair covers both PSUM banks
    ([128, 2, 512]) -> halves ScalarE's ~300cyc/instr overhead.
  * V^T -> V repartition via the DMA transpose XBAR on the sync queue
    (InstDmaTransposeAnt), freeing ~6us of PE time.
  * Transpose-free epilogue: output is y^T [64, 2048] (host transposes);
    normalization = reciprocal of the denominator row (from the ones
    column in V) broadcast across partitions (gpsimd) + one DVE multiply.
  * PSUM: 2x2-bank score tiles + 2 proj banks + 2 AV accumulators = 8.
"""
import os

import numpy as np
import ml_dtypes

import concourse.bass as bass
import concourse.mybir as mybir
import concourse.tile as tile
from concourse import bacc
from concourse.bass_utils import run_bass_kernel_spmd
from concourse.masks import make_identity

P = 128
B, T, C, H = 4, 4096, 1024, 64
TQ = T // 2          # queries per core
CH = 512             # q-chunk width
NCH = TQ // CH       # 4 q-chunks
CT = C // P          # 8 contraction tiles
TC = T // CH         # 8 t-chunks for K/V proj
NKT = T // P         # 32 k-tiles
N_CORES = 8
WAVES = (512, 512, 1024, 1024, 1024)   # consumption-ordered xt column waves

F32 = mybir.dt.float32
BF16 = mybir.dt.bfloat16
FP8 = mybir.dt.float8e4
Exp = mybir.ActivationFunctionType.Exp
MULT = mybir.AluOpType.mult

LAST_EXEC_TIME_NS = None
_COMPILED = None


def _build_graph():
    nc = bacc.Bacc("TRN2", target_bir_lowering=False, debug=False,
                   num_devices=N_CORES)
    xw_d = [nc.dram_tensor(f"xt{w}", [P, CT * WAVES[w]], BF16,
                           kind="ExternalInput").ap()
            for w in range(len(WAVES))]
    wqq = nc.dram_tensor("wqq", [P, CT * P], BF16, kind="ExternalInput").ap()
    wkv = nc.dram_tensor("wkv", [P, CT * P], BF16, kind="ExternalInput").ap()
    mtri = nc.dram_tensor("mtri", [P, P], BF16, kind="ExternalInput").ap()
    dbias = nc.dram_tensor("dbias", [P, 1], F32, kind="ExternalInput").ap()
    y = nc.dram_tensor("y", [H, TQ], F32, kind="ExternalOutput").ap()


    # wave/chunk address maps --------------------------------------------
    wave_of_t = []           # per 512-wide t-chunk: (wave idx, local col0)
    col = 0
    for w, wlen in enumerate(WAVES):
        for lc in range(0, wlen, CH):
            wave_of_t.append((w, lc))
        col += wlen
    assert len(wave_of_t) == TC

    with tile.TileContext(nc) as tc:
        with (
            tc.tile_pool(name="const", bufs=1) as const,
            tc.tile_pool(name="ssb", bufs=6) as sspool,
            tc.tile_pool(name="epi", bufs=2) as epool,
            tc.tile_pool(name="pproj", bufs=2, space="PSUM") as ppool,
            tc.tile_pool(name="ps", bufs=2, space="PSUM") as spool,
            tc.tile_pool(name="po", bufs=1, space="PSUM") as opool,
            tc.tile_pool(name="pt", bufs=1, space="PSUM") as tpool,
        ):
            # ---- constants / persistent tiles ----
            wqq_sb = const.tile([P, CT, P], BF16, name="wqq_sb")
            wkv_sb = const.tile([P, CT, P], BF16, name="wkv_sb")
            mask_sb = const.tile([P, P], BF16, name="mask_sb")
            dbias_sb = const.tile([P, 1], F32, name="dbias_sb")
            warm_sb = const.tile([P, CH], BF16, name="warm_sb")
            ident16 = const.tile([P, P], BF16, name="ident16")
            scratch = const.tile([P, 1], F32, name="scratch")

            xw = [const.tile([P, CT, WAVES[w]], BF16, name=f"xw{w}")
                  for w in range(len(WAVES))]
            qt_sb = const.tile([P, TQ], BF16, name="qt_sb")     # Q^T dup rows
            kt_sb = const.tile([P, T], BF16, name="kt_sb")      # K^T dup halves
            vt_sb = const.tile([64, T], BF16, name="vt_sb")     # V^T
            v_sb = const.tile([P, NKT, 80], BF16, name="v_sb")  # 80: keeps DMA-transpose dst offsets 32B-aligned

            # consts FIRST on the sync ring (so the weights are not starved
            # behind the 8MB xt flood), then the xt waves, same ring.
            nc.sync.dma_start(wkv_sb[:], wkv.rearrange("p (c m) -> p c m", m=P))
            nc.sync.dma_start(
                xw[0][:], xw_d[0].rearrange("p (c t) -> p c t", t=WAVES[0]))
            nc.sync.dma_start(wqq_sb[:], wqq.rearrange("p (c m) -> p c m", m=P))
            nc.sync.dma_start(mask_sb[:], mtri)
            nc.sync.dma_start(dbias_sb[:], dbias)
            nc.gpsimd.memset(v_sb[:, :, H:H + 1], 1.0)
            nc.vector.memset(warm_sb[:], 0.0)
            make_identity(nc, ident16[:])
            # preload the exp table set immediately (scratch <- exp(0))
            nc.vector.memset(scratch[:], 0.0)
            nc.scalar.activation(scratch[:], scratch[:], Exp)
            nc.sync.dma_start(
                xw[1][:], xw_d[1].rearrange("p (c t) -> p c t", t=WAVES[1]))
            # waves 2-4 chain on the (otherwise light) gpsimd queue: a
            # 2-byte dummy read of the previous wave delays each issue until
            # that wave is done -- queued DMAs on a ring progress
            # line-round-robin, so unserialized waves all finish late.
            # the dummy WRITES INTO xw[w] so the big DMA has a real WAW
            # dependency on it -- the Tile scheduler reorders queue
            # instructions by priority, so emission order alone is no fence.
            for w in range(2, len(WAVES)):
                nc.gpsimd.dma_start(xw[w][0:1, 0, 0:1],
                                    xw[w - 1][0:1, 0, 0:1])
                nc.gpsimd.dma_start(
                    xw[w][:], xw_d[w].rearrange("p (c t) -> p c t",
                                                t=WAVES[w]))

            # ---- PE warmup: keep HAM at 8/8 through the DMA wait.
            # N=512 back-to-back (~90% PE duty) -- N=128 warmups never
            # tripped the activity monitor. ----
            for i in range(18):
                wps = ppool.tile([P, CH], F32, tag="ps_proj", name="wps")
                nc.tensor.matmul(wps[:], lhsT=warm_sb[:, 0:P], rhs=warm_sb[:],
                                 start=True, stop=True)

            # ---- helpers ----
            def kv_rhs(t_i, c):
                w, lc = wave_of_t[t_i]
                return xw[w][:, c, lc:lc + CH]

            def q_view(w, c):
                # odd 128-blocks of wave w, c-tile c: [128, nblk, 128]
                wlen = WAVES[w]
                return xw[w][:, c, :].rearrange(
                    "p (hb two q) -> p hb two q", two=2, q=P)[:, :, 1, :]

            def q_proj_units(qc):
                ps = ppool.tile([P, CH], F32, tag="ps_proj", name="ps_q")
                if qc == 0:
                    # chunk 0 queries straddle waves 0 and 1
                    for c in range(CT):
                        yield lambda c=c, ps=ps: nc.tensor.matmul(
                            ps[:, 0:256], lhsT=wqq_sb[:, c, :],
                            rhs=q_view(0, c),
                            start=(c == 0), stop=(c == CT - 1))
                    for c in range(CT):
                        yield lambda c=c, ps=ps: nc.tensor.matmul(
                            ps[:, 256:512], lhsT=wqq_sb[:, c, :],
                            rhs=q_view(1, c),
                            start=(c == 0), stop=(c == CT - 1))
                else:
                    for c in range(CT):
                        yield lambda c=c, ps=ps: nc.tensor.matmul(
                            ps[:], lhsT=wqq_sb[:, c, :],
                            rhs=q_view(qc + 1, c),
                            start=(c == 0), stop=(c == CT - 1))
                yield lambda ps=ps: nc.vector.tensor_copy(
                    qt_sb[:, bass.ts(qc, CH)], ps[:])

            def kv_core_units(t_i):
                ps = ppool.tile([P, CH], F32, tag="ps_proj", name="ps_kv")
                for c in range(CT):
                    yield lambda c=c, ps=ps: nc.tensor.matmul(
                        ps[:], lhsT=wkv_sb[:, c, :], rhs=kv_rhs(t_i, c),
                        start=(c == 0), stop=(c == CT - 1))

                def evac_k(ps=ps):
                    # K^T -> rows 64:128 directly, then SBUF->SBUF DMA dup
                    # into rows 0:64 (for the S^T row-group-0 tiles)
                    nc.vector.tensor_copy(kt_sb[64:128, bass.ts(t_i, CH)],
                                          ps[64:128, :])
                    nc.sync.dma_start(kt_sb[0:64, bass.ts(t_i, CH)],
                                       kt_sb[64:128, bass.ts(t_i, CH)])
                yield evac_k

                def evac_v(ps=ps):
                    nc.vector.tensor_copy(vt_sb[:, bass.ts(t_i, CH)],
                                          ps[0:64, :])
                yield evac_v
                for j in range(CH // P):
                    def vtile(j=j):
                        kt = t_i * (CH // P) + j
                        pt = tpool.tile([P, P], BF16, tag="tr", name="pt")
                        nc.tensor.transpose(pt[:, 0:64], vt_sb[:, bass.ts(kt, P)],
                                            ident16[0:64, 0:64])
                        nc.vector.tensor_copy(v_sb[:, kt, 0:H], pt[:, 0:64])
                    yield vtile

            # ---- attention: flat pipeline over (chunk, k-tile pair) ----
            DEPTH_P = 2  # score pairs in flight (2 PSUM banks each)

            def emit_st(ch, j):
                # pair j covers k-tiles 2j, 2j+1; r0 = first visible q-block
                r0 = max(0, j - 4 * ch)
                ps2 = spool.tile([P, 2, CH], F32, name="ps2")
                s_sb = sspool.tile([P, 2, CH], BF16, tag="s_sb", name="s_sb")
                lo = 64 if (ch == 0 and j < 2) else 0  # first pairs: skip
                nc.tensor.matmul(                        # the kt-dup DMA wait
                    ps2[:, 0, r0 * P:CH],
                    lhsT=kt_sb[lo:lo + 64, bass.ts(2 * j, P)],
                    rhs=qt_sb[lo:lo + 64, ch * CH + r0 * P:(ch + 1) * CH],
                    start=True, stop=True)
                nc.tensor.matmul(
                    ps2[:, 1, r0 * P:CH],
                    lhsT=kt_sb[64:128, bass.ts(2 * j + 1, P)],
                    rhs=qt_sb[64:128, ch * CH + r0 * P:(ch + 1) * CH],
                    start=True, stop=True)
                return ps2, s_sb, r0

            def epilogue_units(ch, po):
                # evacuate po promptly (releases the single AV bank), then
                # normalize from SBUF. Partition-aligned throughout: the
                # denominator row lives at partition 64 (V ones column); DVE
                # lanes cannot cross partitions and partition_broadcast reads
                # partition 0, so a gpsimd DMA moves the reciprocal row.
                osb = epool.tile([P, CH], F32, tag="osb", name="osb")
                tmp = epool.tile([P, CH], F32, tag="tmp", name="tmp")
                tmp2 = epool.tile([P, CH], F32, tag="tmp2", name="tmp2")
                rec0 = epool.tile([1, CH], F32, tag="rec0", name="rec0")
                recb = epool.tile([64, CH], F32, tag="recb", name="recb")
                osc = epool.tile([64, CH], F32, tag="osc", name="osc")

                def evac():
                    nc.vector.tensor_copy(osb[0:H + 1, :], po[0:H + 1, :])
                yield evac

                def normalize():
                    # inline Newton reciprocal of the denominator row (the
                    # RECIPROCAL instr costs ~6.5ns per FREE element -> 3.3us
                    # on a 512-wide row; 6 plain DVE ALU ops cost ~1.8us and
                    # reach ~2e-3 rel accuracy). Bit-trick seed: ~bits(x)
                    # flips the exponent; two NR passes refine.
                    x = osb[64:65, :]
                    y = tmp[64:65, :]
                    t = tmp2[64:65, :]
                    nc.vector.tensor_scalar_sub(x, x, dbias_sb[64:65, 0:1])
                    nc.vector.tensor_tensor(
                        y.bitcast(mybir.dt.uint32), x.bitcast(mybir.dt.uint32),
                        x.bitcast(mybir.dt.uint32), mybir.AluOpType.bitwise_not)
                    nc.vector.tensor_scalar_mul(y, y, -0.23549792)
                    nc.vector.tensor_tensor(t, x, y, MULT)
                    nc.vector.tensor_scalar(t, t, -1.0, 2.0017324,
                                            mybir.AluOpType.mult,
                                            mybir.AluOpType.add)
                    nc.vector.tensor_tensor(y, y, t, MULT)      # y1 (~0.4%)
                    nc.sync.dma_start(rec0[0:1, :], y)
                    nc.gpsimd.partition_broadcast(recb[:], rec0[0:1, :])
                yield normalize

                def store():
                    nc.vector.tensor_tensor(osc[:], osb[0:64, :], recb[:], MULT)
                    nc.sync.dma_start(y[:, bass.ts(ch, CH)], osc[:])
                yield store

            # ---- pre-work (PE queue order): KV0, Q0, KV1 ----
            for u in kv_core_units(0):
                u()
            for u in q_proj_units(0):
                u()
            for u in kv_core_units(1):
                u()

            flat = [(ch, j) for ch in range(NCH) for j in range(4 * ch + 4)]
            # q(ch+1) MUST be fully emitted before the cross-chunk S^T primes
            # (the last DEPTH_P steps of ch): emission order is program order,
            # a later-emitted projection does NOT order before an
            # earlier-emitted reader. kv chunks follow (their k-tiles are
            # first primed 2c steps into the consuming chunk).
            feeds = {
                0: (5, lambda: list(q_proj_units(1)) + list(kv_core_units(2))),
                1: (5, lambda: (list(kv_core_units(3)) + list(q_proj_units(2))
                                + list(kv_core_units(4)))),
                2: (4, lambda: (list(kv_core_units(5)) + list(q_proj_units(3))
                                + list(kv_core_units(6)))),
                3: (2, lambda: list(kv_core_units(7))),
            }

            pending = {}
            for i in range(DEPTH_P):
                pending[flat[i]] = emit_st(*flat[i])
            carry = []
            po = None
            feeder = iter(())
            per_step = 1
            for i, (ch, j) in enumerate(flat):
                n_pairs = 4 * ch + 4
                if j == 0:
                    po = opool.tile([P, CH], F32, name="po")
                    per_step, mk = feeds[ch]
                    feeder = iter(mk() + carry)
                    carry = []
                if i + DEPTH_P < len(flat):
                    pending[flat[i + DEPTH_P]] = emit_st(*flat[i + DEPTH_P])
                ps2, s_sb, r0 = pending.pop((ch, j))
                nc.scalar.activation(s_sb[:, :, r0 * P:CH],
                                     ps2[:, :, r0 * P:CH], Exp, scale=0.125)
                if j >= 4 * ch:  # diagonal block on the odd tile
                    r = j - 4 * ch
                    blk = s_sb[:, 1, r * P:(r + 1) * P]
                    nc.vector.tensor_tensor(blk, blk, mask_sb[:], MULT)
                nc.tensor.matmul(po[0:H + 1, r0 * P:CH],
                                 lhsT=v_sb[:, 2 * j, 0:H + 1],
                                 rhs=s_sb[:, 0, r0 * P:CH],
                                 start=(j == 0), stop=False)
                nc.tensor.matmul(po[0:H + 1, r0 * P:CH],
                                 lhsT=v_sb[:, 2 * j + 1, 0:H + 1],
                                 rhs=s_sb[:, 1, r0 * P:CH],
                                 start=False, stop=(j == n_pairs - 1))
                for _ in range(per_step):
                    u = next(feeder, None)
                    if u is None:
                        break
                    u()
                if j == n_pairs - 1:
                    for u in feeder:
                        u()
                    epi = list(epilogue_units(ch, po))
                    if ch + 1 < NCH:
                        # evac first (releases the single AV bank), the
                        # normalize/store chain AFTER the next chunk's feed
                        # so its DVE ops don't block the feed's CASTs
                        epi[0]()
                        carry = epi[1:]
                    else:
                        for u in epi:
                            u()

    nc.compile()
    return nc


def _shard_inputs(x, Wq, Wk, Wv):
    bf = ml_dtypes.bfloat16
    tri = np.tril(np.ones((P, P), dtype=np.float32)).T  # [kk,qq]=1 iff kk<=qq
    wqq_full = np.concatenate([Wq.T, Wq.T], axis=1)      # [C, 128]
    wkv_full = np.concatenate([Wv.T, Wk.T], axis=1)      # [C, 128]

    def swizzle_cm(a):  # [C, M] -> [128, CT*M] with c-tile-major free dim
        M = a.shape[1]
        return np.ascontiguousarray(
            a.reshape(CT, P, M).transpose(1, 0, 2).reshape(P, CT * M)
        ).astype(bf)

    wqq_s = swizzle_cm(wqq_full)
    wkv_s = swizzle_cm(wkv_full)
    mtri_s = tri.astype(bf)

    in_maps = []
    for core in range(N_CORES):
        b, p = core // 2, core % 2
        if p == 0:
            xt_full = np.concatenate(
                [np.zeros((P, C), dtype=np.float32), x[b][:T - P]], axis=0).T
        else:
            xt_full = x[b].T                              # [C, T]
        sw = xt_full.reshape(CT, P, T).transpose(1, 0, 2)  # [128, CT, T]
        m = {"wqq": wqq_s, "wkv": wkv_s, "mtri": mtri_s,
             "dbias": np.full((P, 1), 128.0 if p == 0 else 0.0,
                              dtype=np.float32)}
        col = 0
        for w, wlen in enumerate(WAVES):
            m[f"xt{w}"] = np.ascontiguousarray(
                sw[:, :, col:col + wlen].reshape(P, CT * wlen)).astype(bf)
            col += wlen
        in_maps.append(m)
    return in_maps


def _unshard(results):
    y = np.zeros((B, T, H), dtype=np.float32)
    for core in range(N_CORES):
        b, p = core // 2, core % 2
        yc = results[core]["y"]                           # [64, 2048]
        for j in range(16):
            g = 2 * j + p
            y[b, P * g:P * g + P] = yc[:, P * j:P * j + P].T
    return y


def kernel(x, Wq, Wk, Wv):
    global LAST_EXEC_TIME_NS, _COMPILED
    x = np.asarray(x, dtype=np.float32)
    Wq = np.asarray(Wq, dtype=np.float32)
    Wk = np.asarray(Wk, dtype=np.float32)
    Wv = np.asarray(Wv, dtype=np.float32)

    if _COMPILED is None:
        _COMPILED = _build_graph()
    nc = _COMPILED

    in_maps = _shard_inputs(x, Wq, Wk, Wv)
    kwargs = {}
    if os.environ.get("ATTN_TRACE"):
        kwargs["trace"] = True
        if os.environ.get("ATTN_TRACE_DIR"):
            kwargs["tmpdir"] = os.environ["ATTN_TRACE_DIR"]
    res = run_bass_kernel_spmd(nc, in_maps, core_ids=list(range(N_CORES)), **kwargs)
    LAST_EXEC_TIME_NS = res.exec_time_ns
    return _unshard(res.results)


# revision 18
# speedup vs baseline: 1.0399x; 1.0399x over previous
"""Single-head causal attention on 8 Trainium2 NeuronCores (Bass/Tile).

Problem: x [4, 4096, 1024] f32, Wq/Wk/Wv [64, 1024] f32 ->
         softmax(causal(q k^T * H^-0.5)) v   -> [4, 4096, 64] f32

Sharding: core = (batch b, parity p), b = core//2, p = core%2. Each core owns
the global 128-wide query tiles g = 2j+p (j=0..15) of its batch -- the parity
interleave balances causal work AND keeps the compiled graph identical across
all 8 cores (SPMD: one NEFF). All parity differences live in host-prepared
data (p=0 gets a zero-padded shifted x; a dbias input removes the pad's
exp(0)*128 contribution to every softmax denominator).

v2 design (from baseline trace analysis: 98.8us, PE 71% busy incl 19us
HAM-throttled, ScalarE 52us of ACTIVATE over 81 calls, 25us before the
first score exp, 12us serial epilogue tail):

  * Host pre-swizzles x^T per core into 5 wave-major DRAM tensors
    ([128, CT*wlen], 8KB contiguous lines) -> ONE DMA issue per wave
    (~0.7us) instead of 32 serialized 607ns issues; consumption-ordered.
  * ~40 warmup matmuls on a zero tile keep the PE HAM activity monitor
    at full clock (2.4GHz) across the initial DMA wait.
  * S^T dual-issue: the score matmul contracts over only H=64 features,
    so two k-tiles run CONCURRENTLY in the two 64-row halves of the PE
    array (tile_position row groups 0/64). K^T is duplicated into both
    partition halves (direct DVE evac to rows 64:128 + SBUF->SBUF DMA to
    rows 0:64); Q^T was already duplicated. 2x S^T throughput, exact.
  * Paired exp: one ACTIVATE per pair covers both PSUM banks
    ([128, 2, 512]) -> halves ScalarE's ~300cyc/instr overhead.
  * V^T -> V repartition via the DMA transpose XBAR on the sync queue
    (InstDmaTransposeAnt), freeing ~6us of PE time.
  * Transpose-free epilogue: output is y^T [64, 2048] (host transposes);
    normalization = reciprocal of the denominator row (from the ones
    column in V) broadcast across partitions (gpsimd) + one DVE multiply.
  * PSUM: 2x2-bank score tiles + 2 proj banks + 2 AV accumulators = 8.
"""
import os

import numpy as np
import ml_dtypes

import concourse.bass as bass
import concourse.mybir as mybir
import concourse.tile as tile
from concourse import bacc
from concourse.bass_utils import run_bass_kernel_spmd
from concourse.masks import make_identity

P = 128
B, T, C, H = 4, 4096, 1024, 64
TQ = T // 2          # queries per core
CH = 512             # q-chunk width
NCH = TQ // CH       # 4 q-chunks
CT = C // P          # 8 contraction tiles
TC = T // CH         # 8 t-chunks for K/V proj
NKT = T // P         # 32 k-tiles
N_CORES = 8
WAVES = (512, 512, 1024, 1024, 1024)   # consumption-ordered xt column waves

F32 = mybir.dt.float32
BF16 = mybir.dt.bfloat16
FP8 = mybir.dt.float8e4
Exp = mybir.ActivationFunctionType.Exp
MULT = mybir.AluOpType.mult

LAST_EXEC_TIME_NS = None
_COMPILED = None


def _build_graph():
    nc = bacc.Bacc("TRN2", target_bir_lowering=False, debug=False,
                   num_devices=N_CORES)
    xw_d = [nc.dram_tensor(f"xt{w}", [P, CT * WAVES[w]], BF16,
                           kind="ExternalInput").ap()
            for w in range(len(WAVES))]
    wqq = nc.dram_tensor("wqq", [P, CT * P], BF16, kind="ExternalInput").ap()
    wkv = nc.dram_tensor("wkv", [P, CT * P], BF16, kind="ExternalInput").ap()
    mtri = nc.dram_tensor("mtri", [P, P], BF16, kind="ExternalInput").ap()
    dbias = nc.dram_tensor("dbias", [P, 1], F32, kind="ExternalInput").ap()
    y = nc.dram_tensor("y", [H, TQ], F32, kind="ExternalOutput").ap()


    # wave/chunk address maps --------------------------------------------
    wave_of_t = []           # per 512-wide t-chunk: (wave idx, local col0)
    col = 0
    for w, wlen in enumerate(WAVES):
        for lc in range(0, wlen, CH):
            wave_of_t.append((w, lc))
        col += wlen
    assert len(wave_of_t) == TC

    with tile.TileContext(nc) as tc:
        with (
            tc.tile_pool(name="const", bufs=1) as const,
            tc.tile_pool(name="ssb", bufs=6) as sspool,
            tc.tile_pool(name="epi", bufs=2) as epool,
            tc.tile_pool(name="pproj", bufs=2, space="PSUM") as ppool,
            tc.tile_pool(name="ps", bufs=2, space="PSUM") as spool,
            tc.tile_pool(name="po", bufs=1, space="PSUM") as opool,
            tc.tile_pool(name="pt", bufs=1, space="PSUM") as tpool,
        ):
            # ---- constants / persistent tiles ----
            wqq_sb = const.tile([P, CT, P], BF16, name="wqq_sb")
            wkv_sb = const.tile([P, CT, P], BF16, name="wkv_sb")
            mask_sb = const.tile([P, P], BF16, name="mask_sb")
            dbias_sb = const.tile([P, 1], F32, name="dbias_sb")
            warm_sb = const.tile([P, CH], BF16, name="warm_sb")
            ident16 = const.tile([P, P], BF16, name="ident16")
            scratch = const.tile([P, 1], F32, name="scratch")

            xw = [const.tile([P, CT, WAVES[w]], BF16, name=f"xw{w}")
                  for w in range(len(WAVES))]
            qt_sb = const.tile([P, TQ], BF16, name="qt_sb")     # Q^T dup rows
            kt_sb = const.tile([P, T], BF16, name="kt_sb")      # K^T dup halves
            vt_sb = const.tile([64, T], BF16, name="vt_sb")     # V^T
            v_sb = const.tile([P, NKT, 80], BF16, name="v_sb")  # 80: keeps DMA-transpose dst offsets 32B-aligned

            # consts FIRST on the sync ring (so the weights are not starved
            # behind the 8MB xt flood), then the xt waves, same ring.
            nc.sync.dma_start(wkv_sb[:], wkv.rearrange("p (c m) -> p c m", m=P))
            nc.sync.dma_start(
                xw[0][:], xw_d[0].rearrange("p (c t) -> p c t", t=WAVES[0]))
            nc.sync.dma_start(wqq_sb[:], wqq.rearrange("p (c m) -> p c m", m=P))
            nc.sync.dma_start(mask_sb[:], mtri)
            nc.sync.dma_start(dbias_sb[:], dbias)
            nc.gpsimd.memset(v_sb[:, :, H:H + 1], 1.0)
            nc.vector.memset(warm_sb[:], 0.0)
            make_identity(nc, ident16[:])
            # preload the exp table set immediately (scratch <- exp(0))
            nc.vector.memset(scratch[:], 0.0)
            nc.scalar.activation(scratch[:], scratch[:], Exp)
            nc.sync.dma_start(
                xw[1][:], xw_d[1].rearrange("p (c t) -> p c t", t=WAVES[1]))
            # waves 2-4 chain on the (otherwise light) gpsimd queue: a
            # 2-byte dummy read of the previous wave delays each issue until
            # that wave is done -- queued DMAs on a ring progress
            # line-round-robin, so unserialized waves all finish late.
            # remaining waves immediately after, same queue: each DMA
            # engine processes its queued lines FIFO, so same-queue issue
            # order gives in-order arrival at full bandwidth (~410 GB/s
            # measured) with no serialization chain needed.
            for w in range(2, len(WAVES)):
                nc.sync.dma_start(
                    xw[w][:], xw_d[w].rearrange("p (c t) -> p c t",
                                                t=WAVES[w]))

            # ---- PE warmup: keep HAM at 8/8 through the DMA wait.
            # N=512 back-to-back (~90% PE duty) -- N=128 warmups never
            # tripped the activity monitor. ----
            for i in range(18):
                wps = ppool.tile([P, CH], F32, tag="ps_proj", name="wps")
                nc.tensor.matmul(wps[:], lhsT=warm_sb[:, 0:P], rhs=warm_sb[:],
                                 start=True, stop=True)

            # ---- helpers ----
            def kv_rhs(t_i, c):
                w, lc = wave_of_t[t_i]
                return xw[w][:, c, lc:lc + CH]

            def q_view(w, c):
                # odd 128-blocks of wave w, c-tile c: [128, nblk, 128]
                wlen = WAVES[w]
                return xw[w][:, c, :].rearrange(
                    "p (hb two q) -> p hb two q", two=2, q=P)[:, :, 1, :]

            def q_proj_units(qc):
                ps = ppool.tile([P, CH], F32, tag="ps_proj", name="ps_q")
                if qc == 0:
                    # chunk 0 queries straddle waves 0 and 1
                    for c in range(CT):
                        yield lambda c=c, ps=ps: nc.tensor.matmul(
                            ps[:, 0:256], lhsT=wqq_sb[:, c, :],
                            rhs=q_view(0, c),
                            start=(c == 0), stop=(c == CT - 1))
                    for c in range(CT):
                        yield lambda c=c, ps=ps: nc.tensor.matmul(
                            ps[:, 256:512], lhsT=wqq_sb[:, c, :],
                            rhs=q_view(1, c),
                            start=(c == 0), stop=(c == CT - 1))
                else:
                    for c in range(CT):
                        yield lambda c=c, ps=ps: nc.tensor.matmul(
                            ps[:], lhsT=wqq_sb[:, c, :],
                            rhs=q_view(qc + 1, c),
                            start=(c == 0), stop=(c == CT - 1))
                yield lambda ps=ps: nc.vector.tensor_copy(
                    qt_sb[:, bass.ts(qc, CH)], ps[:])

            def kv_core_units(t_i):
                ps = ppool.tile([P, CH], F32, tag="ps_proj", name="ps_kv")
                for c in range(CT):
                    yield lambda c=c, ps=ps: nc.tensor.matmul(
                        ps[:], lhsT=wkv_sb[:, c, :], rhs=kv_rhs(t_i, c),
                        start=(c == 0), stop=(c == CT - 1))

                def evac_k(ps=ps):
                    # K^T -> rows 64:128 directly, then SBUF->SBUF DMA dup
                    # into rows 0:64 (for the S^T row-group-0 tiles)
                    nc.vector.tensor_copy(kt_sb[64:128, bass.ts(t_i, CH)],
                                          ps[64:128, :])
                    nc.gpsimd.dma_start(kt_sb[0:64, bass.ts(t_i, CH)],
                                         kt_sb[64:128, bass.ts(t_i, CH)])
                yield evac_k

                def evac_v(ps=ps):
                    nc.vector.tensor_copy(vt_sb[:, bass.ts(t_i, CH)],
                                          ps[0:64, :])
                yield evac_v
                for j in range(CH // P):
                    def vtile(j=j):
                        kt = t_i * (CH // P) + j
                        pt = tpool.tile([P, P], BF16, tag="tr", name="pt")
                        nc.tensor.transpose(pt[:, 0:64], vt_sb[:, bass.ts(kt, P)],
                                            ident16[0:64, 0:64])
                        nc.vector.tensor_copy(v_sb[:, kt, 0:H], pt[:, 0:64])
                    yield vtile

            # ---- attention: flat pipeline over (chunk, k-tile pair) ----
            DEPTH_P = 2  # score pairs in flight (2 PSUM banks each)

            def emit_st(ch, j):
                # pair j covers k-tiles 2j, 2j+1; r0 = first visible q-block
                r0 = max(0, j - 4 * ch)
                ps2 = spool.tile([P, 2, CH], F32, name="ps2")
                s_sb = sspool.tile([P, 2, CH], BF16, tag="s_sb", name="s_sb")
                lo = 64 if (ch == 0 and j < 2) else 0  # first pairs: skip
                nc.tensor.matmul(                        # the kt-dup DMA wait
                    ps2[:, 0, r0 * P:CH],
                    lhsT=kt_sb[lo:lo + 64, bass.ts(2 * j, P)],
                    rhs=qt_sb[lo:lo + 64, ch * CH + r0 * P:(ch + 1) * CH],
                    start=True, stop=True)
                nc.tensor.matmul(
                    ps2[:, 1, r0 * P:CH],
                    lhsT=kt_sb[64:128, bass.ts(2 * j + 1, P)],
                    rhs=qt_sb[64:128, ch * CH + r0 * P:(ch + 1) * CH],
                    start=True, stop=True)
                return ps2, s_sb, r0

            def epilogue_units(ch, po):
                # evacuate po promptly (releases the single AV bank), then
                # normalize from SBUF. Partition-aligned throughout: the
                # denominator row lives at partition 64 (V ones column); DVE
                # lanes cannot cross partitions and partition_broadcast reads
                # partition 0, so a gpsimd DMA moves the reciprocal row.
                osb = epool.tile([P, CH], F32, tag="osb", name="osb")
                tmp = epool.tile([P, CH], F32, tag="tmp", name="tmp")
                tmp2 = epool.tile([P, CH], F32, tag="tmp2", name="tmp2")
                rec0 = epool.tile([1, CH], F32, tag="rec0", name="rec0")
                recb = epool.tile([64, CH], F32, tag="recb", name="recb")
                osc = epool.tile([64, CH], F32, tag="osc", name="osc")

                def evac():
                    nc.vector.tensor_copy(osb[0:H + 1, :], po[0:H + 1, :])
                yield evac

                def normalize():
                    # inline Newton reciprocal of the denominator row (the
                    # RECIPROCAL instr costs ~6.5ns per FREE element -> 3.3us
                    # on a 512-wide row; 6 plain DVE ALU ops cost ~1.8us and
                    # reach ~2e-3 rel accuracy). Bit-trick seed: ~bits(x)
                    # flips the exponent; two NR passes refine.
                    x = osb[64:65, :]
                    y = tmp[64:65, :]
                    t = tmp2[64:65, :]
                    nc.vector.tensor_scalar_sub(x, x, dbias_sb[64:65, 0:1])
                    nc.vector.tensor_tensor(
                        y.bitcast(mybir.dt.uint32), x.bitcast(mybir.dt.uint32),
                        x.bitcast(mybir.dt.uint32), mybir.AluOpType.bitwise_not)
                    nc.vector.tensor_scalar_mul(y, y, -0.23549792)
                    nc.vector.tensor_tensor(t, x, y, MULT)
                    nc.vector.tensor_scalar(t, t, -1.0, 2.0017324,
                                            mybir.AluOpType.mult,
                                            mybir.AluOpType.add)
                    nc.vector.tensor_tensor(y, y, t, MULT)      # y1 (~0.4%)
                    nc.gpsimd.dma_start(rec0[0:1, :], y)
                    nc.gpsimd.partition_broadcast(recb[:], rec0[0:1, :])
                yield normalize

                def store():
                    nc.vector.tensor_tensor(osc[:], osb[0:64, :], recb[:], MULT)
                    nc.gpsimd.dma_start(y[:, bass.ts(ch, CH)], osc[:])
                yield store

            # ---- pre-work (PE queue order): KV0, Q0, KV1 ----
            for u in kv_core_units(0):
                u()
            for u in q_proj_units(0):
                u()
            for u in kv_core_units(1):
                u()

            flat = [(ch, j) for ch in range(NCH) for j in range(4 * ch + 4)]
            # q(ch+1) MUST be fully emitted before the cross-chunk S^T primes
            # (the last DEPTH_P steps of ch): emission order is program order,
            # a later-emitted projection does NOT order before an
            # earlier-emitted reader. kv chunks follow (their k-tiles are
            # first primed 2c steps into the consuming chunk).
            feeds = {
                0: (5, lambda: list(q_proj_units(1)) + list(kv_core_units(2))),
                1: (5, lambda: (list(kv_core_units(3)) + list(q_proj_units(2))
                                + list(kv_core_units(4)))),
                2: (4, lambda: (list(kv_core_units(5)) + list(q_proj_units(3))
                                + list(kv_core_units(6)))),
                3: (2, lambda: list(kv_core_units(7))),
            }

            pending = {}
            for i in range(DEPTH_P):
                pending[flat[i]] = emit_st(*flat[i])
            carry = []
            po = None
            feeder = iter(())
            per_step = 1
            for i, (ch, j) in enumerate(flat):
                n_pairs = 4 * ch + 4
                if j == 0:
                    po = opool.tile([P, CH], F32, name="po")
                    per_step, mk = feeds[ch]
                    feeder = iter(mk() + carry)
                    carry = []
                if i + DEPTH_P < len(flat):
                    pending[flat[i + DEPTH_P]] = emit_st(*flat[i + DEPTH_P])
                ps2, s_sb, r0 = pending.pop((ch, j))
                nc.scalar.activation(s_sb[:, :, r0 * P:CH],
                                     ps2[:, :, r0 * P:CH], Exp, scale=0.125)
                if j >= 4 * ch:  # diagonal block on the odd tile
                    r = j - 4 * ch
                    blk = s_sb[:, 1, r * P:(r + 1) * P]
                    nc.vector.tensor_tensor(blk, blk, mask_sb[:], MULT)
                nc.tensor.matmul(po[0:H + 1, r0 * P:CH],
                                 lhsT=v_sb[:, 2 * j, 0:H + 1],
                                 rhs=s_sb[:, 0, r0 * P:CH],
                                 start=(j == 0), stop=False)
                nc.tensor.matmul(po[0:H + 1, r0 * P:CH],
                                 lhsT=v_sb[:, 2 * j + 1, 0:H + 1],
                                 rhs=s_sb[:, 1, r0 * P:CH],
                                 start=False, stop=(j == n_pairs - 1))
                for _ in range(per_step):
                    u = next(feeder, None)
                    if u is None:
                        break
                    u()
                if j == n_pairs - 1:
                    for u in feeder:
                        u()
                    epi = list(epilogue_units(ch, po))
                    if ch + 1 < NCH:
                        # evac first (releases the single AV bank), the
                        # normalize/store chain AFTER the next chunk's feed
                        # so its DVE ops don't block the feed's CASTs
                        epi[0]()
                        carry = epi[1:]
                    else:
                        for u in epi:
                            u()

    nc.compile()
    return nc


def _shard_inputs(x, Wq, Wk, Wv):
    bf = ml_dtypes.bfloat16
    tri = np.tril(np.ones((P, P), dtype=np.float32)).T  # [kk,qq]=1 iff kk<=qq
    wqq_full = np.concatenate([Wq.T, Wq.T], axis=1)      # [C, 128]
    wkv_full = np.concatenate([Wv.T, Wk.T], axis=1)      # [C, 128]

    def swizzle_cm(a):  # [C, M] -> [128, CT*M] with c-tile-major free dim
        M = a.shape[1]
        return np.ascontiguousarray(
            a.reshape(CT, P, M).transpose(1, 0, 2).reshape(P, CT * M)
        ).astype(bf)

    wqq_s = swizzle_cm(wqq_full)
    wkv_s = swizzle_cm(wkv_full)
    mtri_s = tri.astype(bf)

    in_maps = []
    for core in range(N_CORES):
        b, p = core // 2, core % 2
        if p == 0:
            xt_full = np.concatenate(
                [np.zeros((P, C), dtype=np.float32), x[b][:T - P]], axis=0).T
        else:
            xt_full = x[b].T                              # [C, T]
        sw = xt_full.reshape(CT, P, T).transpose(1, 0, 2)  # [128, CT, T]
        m = {"wqq": wqq_s, "wkv": wkv_s, "mtri": mtri_s,
             "dbias": np.full((P, 1), 128.0 if p == 0 else 0.0,
                              dtype=np.float32)}
        col = 0
        for w, wlen in enumerate(WAVES):
            m[f"xt{w}"] = np.ascontiguousarray(
                sw[:, :, col:col + wlen].reshape(P, CT * wlen)).astype(bf)
            col += wlen
        in_maps.append(m)
    return in_maps


def _unshard(results):
    y = np.zeros((B, T, H), dtype=np.float32)
    for core in range(N_CORES):
        b, p = core // 2, core % 2
        yc = results[core]["y"]                           # [64, 2048]
        for j in range(16):
            g = 2 * j + p
            y[b, P * g:P * g + P] = yc[:, P * j:P * j + P].T
    return y


def kernel(x, Wq, Wk, Wv):
    global LAST_EXEC_TIME_NS, _COMPILED
    x = np.asarray(x, dtype=np.float32)
    Wq = np.asarray(Wq, dtype=np.float32)
    Wk = np.asarray(Wk, dtype=np.float32)
    Wv = np.asarray(Wv, dtype=np.float32)

    if _COMPILED is None:
        _COMPILED = _build_graph()
    nc = _COMPILED

    in_maps = _shard_inputs(x, Wq, Wk, Wv)
    kwargs = {}
    if os.environ.get("ATTN_TRACE"):
        kwargs["trace"] = True
        if os.environ.get("ATTN_TRACE_DIR"):
            kwargs["tmpdir"] = os.environ["ATTN_TRACE_DIR"]
    res = run_bass_kernel_spmd(nc, in_maps, core_ids=list(range(N_CORES)), **kwargs)
    LAST_EXEC_TIME_NS = res.exec_time_ns
    return _unshard(res.results)


# revision 20
# speedup vs baseline: 1.1263x; 1.0830x over previous
"""Single-head causal attention on 8 Trainium2 NeuronCores (Bass/Tile).

Problem: x [4, 4096, 1024] f32, Wq/Wk/Wv [64, 1024] f32 ->
         softmax(causal(q k^T * H^-0.5)) v   -> [4, 4096, 64] f32

Sharding: core = (batch b, parity p), b = core//2, p = core%2. Each core owns
the global 128-wide query tiles g = 2j+p (j=0..15) of its batch -- the parity
interleave balances causal work AND keeps the compiled graph identical across
all 8 cores (SPMD: one NEFF). All parity differences live in host-prepared
data (p=0 gets a zero-padded shifted x; a dbias input removes the pad's
exp(0)*128 contribution to every softmax denominator).

v2 design (from baseline trace analysis: 98.8us, PE 71% busy incl 19us
HAM-throttled, ScalarE 52us of ACTIVATE over 81 calls, 25us before the
first score exp, 12us serial epilogue tail):

  * Host pre-swizzles x^T per core into 5 wave-major DRAM tensors
    ([128, CT*wlen], 8KB contiguous lines) -> ONE DMA issue per wave
    (~0.7us) instead of 32 serialized 607ns issues; consumption-ordered.
  * ~40 warmup matmuls on a zero tile keep the PE HAM activity monitor
    at full clock (2.4GHz) across the initial DMA wait.
  * S^T dual-issue: the score matmul contracts over only H=64 features,
    so two k-tiles run CONCURRENTLY in the two 64-row halves of the PE
    array (tile_position row groups 0/64). K^T is duplicated into both
    partition halves (direct DVE evac to rows 64:128 + SBUF->SBUF DMA to
    rows 0:64); Q^T was already duplicated. 2x S^T throughput, exact.
  * Paired exp: one ACTIVATE per pair covers both PSUM banks
    ([128, 2, 512]) -> halves ScalarE's ~300cyc/instr overhead.
  * V^T -> V repartition via the DMA transpose XBAR on the sync queue
    (InstDmaTransposeAnt), freeing ~6us of PE time.
  * Transpose-free epilogue: output is y^T [64, 2048] (host transposes);
    normalization = reciprocal of the denominator row (from the ones
    column in V) broadcast across partitions (gpsimd) + one DVE multiply.
  * PSUM: 2x2-bank score tiles + 2 proj banks + 2 AV accumulators = 8.
"""
import os

import numpy as np
import ml_dtypes

import concourse.bass as bass
import concourse.mybir as mybir
import concourse.tile as tile
from concourse import bacc
from concourse.bass_utils import run_bass_kernel_spmd
from concourse.masks import make_identity

P = 128
B, T, C, H = 4, 4096, 1024, 64
TQ = T // 2          # queries per core
CH = 512             # q-chunk width
NCH = TQ // CH       # 4 q-chunks
CT = C // P          # 8 contraction tiles
TC = T // CH         # 8 t-chunks for K/V proj
NKT = T // P         # 32 k-tiles
N_CORES = 8
WAVES = (512, 512, 1024, 1024, 1024)   # consumption-ordered xt column waves

F32 = mybir.dt.float32
BF16 = mybir.dt.bfloat16
FP8 = mybir.dt.float8e4
Exp = mybir.ActivationFunctionType.Exp
MULT = mybir.AluOpType.mult

LAST_EXEC_TIME_NS = None
_COMPILED = None


def _build_graph():
    nc = bacc.Bacc("TRN2", target_bir_lowering=False, debug=False,
                   num_devices=N_CORES)
    xw_d = [nc.dram_tensor(f"xt{w}", [P, CT * WAVES[w]], BF16,
                           kind="ExternalInput").ap()
            for w in range(len(WAVES))]
    wqq = nc.dram_tensor("wqq", [P, CT * P], BF16, kind="ExternalInput").ap()
    wkv = nc.dram_tensor("wkv", [P, CT * P], BF16, kind="ExternalInput").ap()
    mtri = nc.dram_tensor("mtri", [P, P], BF16, kind="ExternalInput").ap()
    dbias = nc.dram_tensor("dbias", [P, 1], F32, kind="ExternalInput").ap()
    y = nc.dram_tensor("y", [H, TQ], F32, kind="ExternalOutput").ap()


    # wave/chunk address maps --------------------------------------------
    wave_of_t = []           # per 512-wide t-chunk: (wave idx, local col0)
    col = 0
    for w, wlen in enumerate(WAVES):
        for lc in range(0, wlen, CH):
            wave_of_t.append((w, lc))
        col += wlen
    assert len(wave_of_t) == TC

    with tile.TileContext(nc) as tc:
        with (
            tc.tile_pool(name="const", bufs=1) as const,
            tc.tile_pool(name="ssb", bufs=6) as sspool,
            tc.tile_pool(name="epi", bufs=2) as epool,
            tc.tile_pool(name="pproj", bufs=2, space="PSUM") as ppool,
            tc.tile_pool(name="ps", bufs=2, space="PSUM") as spool,
            tc.tile_pool(name="po", bufs=1, space="PSUM") as opool,
            tc.tile_pool(name="pt", bufs=1, space="PSUM") as tpool,
        ):
            # ---- constants / persistent tiles ----
            wqq_sb = const.tile([P, CT, P], BF16, name="wqq_sb")
            wkv_sb = const.tile([P, CT, P], BF16, name="wkv_sb")
            mask_sb = const.tile([P, P], BF16, name="mask_sb")
            dbias_sb = const.tile([P, 1], F32, name="dbias_sb")
            warm_sb = const.tile([P, CH], BF16, name="warm_sb")
            ident16 = const.tile([P, P], BF16, name="ident16")
            shid = const.tile([P, 64], BF16, name="shid")
            ones1 = const.tile([1, 64], BF16, name="ones1")
            scratch = const.tile([P, 1], F32, name="scratch")

            xw = [const.tile([P, CT, WAVES[w]], BF16, name=f"xw{w}")
                  for w in range(len(WAVES))]
            qt_sb = const.tile([P, TQ], BF16, name="qt_sb")     # Q^T dup rows
            kt_sb = const.tile([P, T], BF16, name="kt_sb")      # K^T dup halves
            vt_sb = const.tile([64, T], BF16, name="vt_sb")     # V^T
            v_sb = const.tile([P, NKT, 80], BF16, name="v_sb")  # 80: keeps DMA-transpose dst offsets 32B-aligned

            # consts FIRST on the sync ring (so the weights are not starved
            # behind the 8MB xt flood), then the xt waves, same ring.
            nc.sync.dma_start(wkv_sb[:], wkv.rearrange("p (c m) -> p c m", m=P))
            nc.sync.dma_start(
                xw[0][:], xw_d[0].rearrange("p (c t) -> p c t", t=WAVES[0]))
            nc.sync.dma_start(wqq_sb[:], wqq.rearrange("p (c m) -> p c m", m=P))
            nc.gpsimd.memset(v_sb[:, :, H:H + 1], 1.0)
            nc.vector.memset(warm_sb[:], 0.0)
            nc.vector.memset(kt_sb[0:64, :], 0.0)
            nc.gpsimd.memset(ones1[:], 1.0)
            make_identity(nc, ident16[:])
            # shifted identity: shid[y+64, y] = 1 (for the K^T partition dup)
            nc.gpsimd.memset(shid[:], 0.0)
            nc.gpsimd.affine_select(
                out=shid[:], in_=shid[:],
                compare_op=mybir.AluOpType.not_equal, fill=1.0,
                base=-64, pattern=[[-1, 64]], channel_multiplier=1)
            # preload the exp table set immediately (scratch <- exp(0))
            nc.vector.memset(scratch[:], 0.0)
            nc.scalar.activation(scratch[:], scratch[:], Exp)
            nc.sync.dma_start(
                xw[1][:], xw_d[1].rearrange("p (c t) -> p c t", t=WAVES[1]))
            nc.sync.dma_start(mask_sb[:], mtri)
            nc.sync.dma_start(dbias_sb[:], dbias)
            # waves 2-4 chain on the (otherwise light) gpsimd queue: a
            # 2-byte dummy read of the previous wave delays each issue until
            # that wave is done -- queued DMAs on a ring progress
            # line-round-robin, so unserialized waves all finish late.
            # remaining waves immediately after, same queue: each DMA
            # engine processes its queued lines FIFO, so same-queue issue
            # order gives in-order arrival at full bandwidth (~410 GB/s
            # measured) with no serialization chain needed.
            for w in range(2, len(WAVES)):
                nc.sync.dma_start(
                    xw[w][:], xw_d[w].rearrange("p (c t) -> p c t",
                                                t=WAVES[w]))

            # ---- PE warmup: keep HAM at 8/8 through the DMA wait.
            # N=512 back-to-back (~90% PE duty) -- N=128 warmups never
            # tripped the activity monitor. ----
            for i in range(18):
                wps = ppool.tile([P, CH], F32, tag="ps_proj", name="wps")
                nc.tensor.matmul(wps[:], lhsT=warm_sb[:, 0:P], rhs=warm_sb[:],
                                 start=True, stop=True)

            # ---- helpers ----
            def kv_rhs(t_i, c):
                w, lc = wave_of_t[t_i]
                return xw[w][:, c, lc:lc + CH]

            def q_view(w, c):
                # odd 128-blocks of wave w, c-tile c: [128, nblk, 128]
                wlen = WAVES[w]
                return xw[w][:, c, :].rearrange(
                    "p (hb two q) -> p hb two q", two=2, q=P)[:, :, 1, :]

            def q_proj_units(qc):
                ps = ppool.tile([P, CH], F32, tag="ps_proj", name="ps_q")
                if qc == 0:
                    # chunk 0 queries straddle waves 0 and 1
                    for c in range(CT):
                        yield lambda c=c, ps=ps: nc.tensor.matmul(
                            ps[:, 0:256], lhsT=wqq_sb[:, c, :],
                            rhs=q_view(0, c),
                            start=(c == 0), stop=(c == CT - 1))
                    for c in range(CT):
                        yield lambda c=c, ps=ps: nc.tensor.matmul(
                            ps[:, 256:512], lhsT=wqq_sb[:, c, :],
                            rhs=q_view(1, c),
                            start=(c == 0), stop=(c == CT - 1))
                else:
                    for c in range(CT):
                        yield lambda c=c, ps=ps: nc.tensor.matmul(
                            ps[:], lhsT=wqq_sb[:, c, :],
                            rhs=q_view(qc + 1, c),
                            start=(c == 0), stop=(c == CT - 1))
                yield lambda ps=ps: nc.vector.tensor_copy(
                    qt_sb[:, bass.ts(qc, CH)], ps[:])

            def kv_core_units(t_i):
                ps = ppool.tile([P, CH], F32, tag="ps_proj", name="ps_kv")
                for c in range(CT):
                    yield lambda c=c, ps=ps: nc.tensor.matmul(
                        ps[:], lhsT=wkv_sb[:, c, :], rhs=kv_rhs(t_i, c),
                        start=(c == 0), stop=(c == CT - 1))

                def evac_k(ps=ps):
                    # K^T -> rows 64:128 directly
                    nc.vector.tensor_copy(kt_sb[64:128, bass.ts(t_i, CH)],
                                          ps[64:128, :])
                yield evac_k

                def dup_k():
                    # partition dup rows 64:128 -> 0:64 via a PE
                    # shift-identity matmul (no DMA: small SBUF->SBUF DMAs
                    # steal round-robin slots from the input waves)
                    psh = ppool.tile([P, CH], F32, tag="ps_proj", name="psh")
                    nc.tensor.matmul(psh[0:64, :], lhsT=shid[:, :],
                                     rhs=kt_sb[:, bass.ts(t_i, CH)],
                                     start=True, stop=True)
                    nc.vector.tensor_copy(kt_sb[0:64, bass.ts(t_i, CH)],
                                          psh[0:64, :])
                yield dup_k

                def evac_v(ps=ps):
                    nc.vector.tensor_copy(vt_sb[:, bass.ts(t_i, CH)],
                                          ps[0:64, :])
                yield evac_v
                for j in range(CH // P):
                    def vtile(j=j):
                        kt = t_i * (CH // P) + j
                        pt = tpool.tile([P, P], BF16, tag="tr", name="pt")
                        nc.tensor.transpose(pt[:, 0:64], vt_sb[:, bass.ts(kt, P)],
                                            ident16[0:64, 0:64])
                        nc.vector.tensor_copy(v_sb[:, kt, 0:H], pt[:, 0:64])
                    yield vtile

            # ---- attention: flat pipeline over (chunk, k-tile pair) ----
            DEPTH_P = 2  # score pairs in flight (2 PSUM banks each)

            def emit_st(ch, j):
                # pair j covers k-tiles 2j, 2j+1; r0 = first visible q-block
                r0 = max(0, j - 4 * ch)
                ps2 = spool.tile([P, 2, CH], F32, name="ps2")
                s_sb = sspool.tile([P, 2, CH], BF16, tag="s_sb", name="s_sb")
                lo = 64 if (ch == 0 and j < 2) else 0  # first pairs: skip
                nc.tensor.matmul(                        # the kt-dup DMA wait
                    ps2[:, 0, r0 * P:CH],
                    lhsT=kt_sb[lo:lo + 64, bass.ts(2 * j, P)],
                    rhs=qt_sb[lo:lo + 64, ch * CH + r0 * P:(ch + 1) * CH],
                    start=True, stop=True)
                nc.tensor.matmul(
                    ps2[:, 1, r0 * P:CH],
                    lhsT=kt_sb[64:128, bass.ts(2 * j + 1, P)],
                    rhs=qt_sb[64:128, ch * CH + r0 * P:(ch + 1) * CH],
                    start=True, stop=True)
                return ps2, s_sb, r0

            def epilogue_units(ch, po):
                # evacuate po promptly (releases the single AV bank), then
                # normalize from SBUF. Partition-aligned throughout: the
                # denominator row lives at partition 64 (V ones column); DVE
                # lanes cannot cross partitions and partition_broadcast reads
                # partition 0, so a gpsimd DMA moves the reciprocal row.
                osb = epool.tile([P, CH], F32, tag="osb", name="osb")
                tmp = epool.tile([P, CH], F32, tag="tmp", name="tmp")
                tmp2 = epool.tile([P, CH], F32, tag="tmp2", name="tmp2")
                rec0 = epool.tile([1, CH], F32, tag="rec0", name="rec0")
                recb = epool.tile([64, CH], F32, tag="recb", name="recb")
                osc = epool.tile([64, CH], F32, tag="osc", name="osc")
                rec0b = epool.tile([1, CH], BF16, tag="rec0b", name="rec0b")
                pre = (opool.tile([P, CH], F32, tag="po", name="pre")
                       if ch + 1 >= NCH else None)

                def evac():
                    nc.vector.tensor_copy(osb[0:H + 1, :], po[0:H + 1, :])
                yield evac

                def normalize():
                    # inline Newton reciprocal of the denominator row (the
                    # RECIPROCAL instr costs ~6.5ns per FREE element -> 3.3us
                    # on a 512-wide row; 6 plain DVE ALU ops cost ~1.8us and
                    # reach ~2e-3 rel accuracy). Bit-trick seed: ~bits(x)
                    # flips the exponent; two NR passes refine.
                    x = osb[64:65, :]
                    y = tmp[64:65, :]
                    t = tmp2[64:65, :]
                    nc.vector.tensor_scalar_sub(x, x, dbias_sb[64:65, 0:1])
                    nc.vector.tensor_tensor(
                        y.bitcast(mybir.dt.uint32), x.bitcast(mybir.dt.uint32),
                        x.bitcast(mybir.dt.uint32), mybir.AluOpType.bitwise_not)
                    nc.vector.tensor_scalar_mul(y, y, -0.23549792)
                    nc.vector.tensor_tensor(t, x, y, MULT)
                    nc.vector.tensor_scalar(t, t, -1.0, 2.0017324,
                                            mybir.AluOpType.mult,
                                            mybir.AluOpType.add)
                    nc.vector.tensor_tensor(y, y, t, MULT)      # y1 (~0.4%)
                    if ch + 1 < NCH:
                        nc.gpsimd.dma_start(rec0[0:1, :], y)
                        nc.gpsimd.partition_broadcast(recb[:], rec0[0:1, :])
                    else:
                        # tail path: PE ones-matmul broadcast into the AV
                        # bank (free now) -- skips two gpsimd round-trips
                        nc.vector.tensor_copy(rec0b[0:1, :], y)
                        nc.tensor.matmul(pre[0:64, :], lhsT=ones1[0:1, :],
                                         rhs=rec0b[0:1, :],
                                         start=True, stop=True)
                yield normalize

                def store():
                    if ch + 1 < NCH:
                        nc.vector.tensor_tensor(osc[:], osb[0:64, :],
                                                recb[:], MULT)
                    else:
                        nc.vector.tensor_tensor(osc[:], osb[0:64, :],
                                                pre[0:64, :], MULT)
                    nc.gpsimd.dma_start(y[:, bass.ts(ch, CH)], osc[:])
                yield store

            # ---- pre-work (PE queue order): KV0, Q0, KV1 ----
            for u in kv_core_units(0):
                u()
            for u in q_proj_units(0):
                u()
            for u in kv_core_units(1):
                u()

            flat = [(ch, j) for ch in range(NCH) for j in range(4 * ch + 4)]
            # q(ch+1) MUST be fully emitted before the cross-chunk S^T primes
            # (the last DEPTH_P steps of ch): emission order is program order,
            # a later-emitted projection does NOT order before an
            # earlier-emitted reader. kv chunks follow (their k-tiles are
            # first primed 2c steps into the consuming chunk).
            feeds = {
                0: (5, lambda: list(q_proj_units(1)) + list(kv_core_units(2))),
                1: (5, lambda: (list(kv_core_units(3)) + list(q_proj_units(2))
                                + list(kv_core_units(4)))),
                2: (4, lambda: (list(kv_core_units(5)) + list(q_proj_units(3))
                                + list(kv_core_units(6)))),
                3: (2, lambda: list(kv_core_units(7))),
            }

            pending = {}
            for i in range(DEPTH_P):
                pending[flat[i]] = emit_st(*flat[i])
            carry = []
            po = None
            feeder = iter(())
            per_step = 1
            for i, (ch, j) in enumerate(flat):
                n_pairs = 4 * ch + 4
                if j == 0:
                    po = opool.tile([P, CH], F32, tag="po", name="po")
                    per_step, mk = feeds[ch]
                    feeder = iter(mk() + carry)
                    carry = []
                if i + DEPTH_P < len(flat):
                    pending[flat[i + DEPTH_P]] = emit_st(*flat[i + DEPTH_P])
                ps2, s_sb, r0 = pending.pop((ch, j))
                nc.scalar.activation(s_sb[:, :, r0 * P:CH],
                                     ps2[:, :, r0 * P:CH], Exp, scale=0.125)
                if j >= 4 * ch:  # diagonal block on the odd tile
                    r = j - 4 * ch
                    blk = s_sb[:, 1, r * P:(r + 1) * P]
                    nc.vector.tensor_tensor(blk, blk, mask_sb[:], MULT)
                nc.tensor.matmul(po[0:H + 1, r0 * P:CH],
                                 lhsT=v_sb[:, 2 * j, 0:H + 1],
                                 rhs=s_sb[:, 0, r0 * P:CH],
                                 start=(j == 0), stop=False)
                nc.tensor.matmul(po[0:H + 1, r0 * P:CH],
                                 lhsT=v_sb[:, 2 * j + 1, 0:H + 1],
                                 rhs=s_sb[:, 1, r0 * P:CH],
                                 start=False, stop=(j == n_pairs - 1))
                for _ in range(per_step):
                    u = next(feeder, None)
                    if u is None:
                        break
                    u()
                if j == n_pairs - 1:
                    for u in feeder:
                        u()
                    epi = list(epilogue_units(ch, po))
                    if ch + 1 < NCH:
                        # evac first (releases the single AV bank), the
                        # normalize/store chain AFTER the next chunk's feed
                        # so its DVE ops don't block the feed's CASTs
                        epi[0]()
                        carry = epi[1:]
                    else:
                        for u in epi:
                            u()

    nc.compile()
    return nc


def _shard_inputs(x, Wq, Wk, Wv):
    bf = ml_dtypes.bfloat16
    tri = np.tril(np.ones((P, P), dtype=np.float32)).T  # [kk,qq]=1 iff kk<=qq
    wqq_full = np.concatenate([Wq.T, Wq.T], axis=1)      # [C, 128]
    wkv_full = np.concatenate([Wv.T, Wk.T], axis=1)      # [C, 128]

    def swizzle_cm(a):  # [C, M] -> [128, CT*M] with c-tile-major free dim
        M = a.shape[1]
        return np.ascontiguousarray(
            a.reshape(CT, P, M).transpose(1, 0, 2).reshape(P, CT * M)
        ).astype(bf)

    wqq_s = swizzle_cm(wqq_full)
    wkv_s = swizzle_cm(wkv_full)
    mtri_s = tri.astype(bf)

    in_maps = []
    for core in range(N_CORES):
        b, p = core // 2, core % 2
        if p == 0:
            xt_full = np.concatenate(
                [np.zeros((P, C), dtype=np.float32), x[b][:T - P]], axis=0).T
        else:
            xt_full = x[b].T                              # [C, T]
        sw = xt_full.reshape(CT, P, T).transpose(1, 0, 2)  # [128, CT, T]
        m = {"wqq": wqq_s, "wkv": wkv_s, "mtri": mtri_s,
             "dbias": np.full((P, 1), 128.0 if p == 0 else 0.0,
                              dtype=np.float32)}
        col = 0
        for w, wlen in enumerate(WAVES):
            m[f"xt{w}"] = np.ascontiguousarray(
                sw[:, :, col:col + wlen].reshape(P, CT * wlen)).astype(bf)
            col += wlen
        in_maps.append(m)
    return in_maps


def _unshard(results):
    y = np.zeros((B, T, H), dtype=np.float32)
    for core in range(N_CORES):
        b, p = core // 2, core % 2
        yc = results[core]["y"]                           # [64, 2048]
        for j in range(16):
            g = 2 * j + p
            y[b, P * g:P * g + P] = yc[:, P * j:P * j + P].T
    return y


def kernel(x, Wq, Wk, Wv):
    global LAST_EXEC_TIME_NS, _COMPILED
    x = np.asarray(x, dtype=np.float32)
    Wq = np.asarray(Wq, dtype=np.float32)
    Wk = np.asarray(Wk, dtype=np.float32)
    Wv = np.asarray(Wv, dtype=np.float32)

    if _COMPILED is None:
        _COMPILED = _build_graph()
    nc = _COMPILED

    in_maps = _shard_inputs(x, Wq, Wk, Wv)
    kwargs = {}
    if os.environ.get("ATTN_TRACE"):
        kwargs["trace"] = True
        if os.environ.get("ATTN_TRACE_DIR"):
            kwargs["tmpdir"] = os.environ["ATTN_TRACE_DIR"]
    res = run_bass_kernel_spmd(nc, in_maps, core_ids=list(range(N_CORES)), **kwargs)
    LAST_EXEC_TIME_NS = res.exec_time_ns
    return _unshard(res.results)
